# revision 1
# baseline (speedup 1.0000x reference)
"""Trainium2 Bass kernel for the controlled-U (CU) gate application.

Math: the reference builds U = P0 (x) I (x) ... + P1 (x) Mexp (x) I ...
with dim=2, wires=12, index=(0,1), control_state=(1,). This factors as

    U = diag(I_2, Mexp) (x) I_1024        (4096 x 4096)

so U @ x is:
    out[0:2048]     = x[0:2048]                        (identity)
    out[2048:3072]  = c00 * x[2048:3072] + c01 * x[3072:4096]
    out[3072:4096]  = c10 * x[2048:3072] + c11 * x[3072:4096]

with [[c00, c01], [c10, c11]] = Mexp = expm(M - M^H), a 2x2 unitary
computed exactly on host (eigendecomposition of the 2x2 Hermitian
generator).

Device strategy (8 NeuronCores, SPMD, row sharding — all DMA runs are
full 4 KiB rows):
  - core d gets top rows [256d, 256d+256) (identity) plus the bottom
    pair rows [2048+128d, +128) and [3072+128d, +128) (the 2x2 mix);
    every core runs the identical program on 1/8 of the work.
  - top rows: DVE strided copies interleave re/im -> complex64 layout
  - bottom pair rows are split by batch column between TensorE (fp32
    matmuls with 32x32 diagonal stationary tiles at concurrent tile
    positions, PSUM accumulation, ACT interleave-copies PSUM -> SBUF)
    and the DVE (fused scalar_tensor_tensor MAC chains with per-
    partition scalar coefficients, writing the interleaved layout
    directly), balanced so both engines hide under the DMA stream
  - outputs per core: f32 rows of interleaved (re, im) pairs; the host
    reassembles the (4096, 2048) f32 buffer and reinterprets it as
    complex64 (zero-copy view).

All arithmetic is fp32 (exact vs the reference up to rounding, ~1e-7).
"""

import numpy as np

import concourse.bacc as bacc
import concourse.mybir as mybir
from concourse.tile import TileContext
from concourse.bass_utils import run_bass_kernel_spmd

# Problem geometry (hardcoded per the task contract).
D = 4096           # state dimension 2**12
B = 1024           # batch
NCORES = 8
P = 128            # SBUF partitions
TROWS = D // 2 // NCORES   # 256 top (identity) rows per core
PROWS = D // 4 // NCORES   # 128 bottom pair rows per core
F32 = mybir.dt.float32

NDIAG = 12         # 12 diagonal coefficient matrices (see _coef_values)
TP = 32            # PE sub-tile size for tile_position concurrency
CH = B // 2        # column half processed per compute engine

# quantity -> (out half, interleave parity, coefficient idx per input).
# inputs are (xr1, xi1, xr2, xi2); coefficients include baked-in signs.
RECIPES = [
    ("o1re", 0, 0, (0, 1, 3, 4)),
    ("o1im", 0, 1, (2, 0, 5, 3)),
    ("o2re", 1, 0, (6, 7, 9, 10)),
    ("o2im", 1, 1, (8, 6, 11, 9)),
]


def _build_nc() -> bacc.Bacc:
    """Build the per-core Bass/Tile program (identical on all 8 cores)."""
    # Bacc (not raw Bass): its compile() lowers multi-dependency sync waits
    # through event semaphores — raw Bass trips walrus's per-instruction
    # wait-slot limit ("Too many sync wait commands").
    nc = bacc.Bacc("TRN2", enable_partition_id=False)

    xr_t = nc.dram_tensor("xr_t", [TROWS, B], F32, kind="ExternalInput")
    xi_t = nc.dram_tensor("xi_t", [TROWS, B], F32, kind="ExternalInput")
    xr_b1 = nc.dram_tensor("xr_b1", [PROWS, B], F32, kind="ExternalInput")
    xi_b1 = nc.dram_tensor("xi_b1", [PROWS, B], F32, kind="ExternalInput")
    xr_b2 = nc.dram_tensor("xr_b2", [PROWS, B], F32, kind="ExternalInput")
    xi_b2 = nc.dram_tensor("xi_b2", [PROWS, B], F32, kind="ExternalInput")
    # coef[p, k*TP + (p % TP)] = value_k  ->  32x32 diagonal blocks.
    coef = nc.dram_tensor("coef", [P, NDIAG * TP], F32, kind="ExternalInput")
    cvec = nc.dram_tensor("cvec", [P, NDIAG], F32, kind="ExternalInput")

    out_t = nc.dram_tensor("out_t", [TROWS, 2 * B], F32, kind="ExternalOutput")
    out_b1 = nc.dram_tensor("out_b1", [PROWS, 2 * B], F32, kind="ExternalOutput")
    out_b2 = nc.dram_tensor("out_b2", [PROWS, 2 * B], F32, kind="ExternalOutput")

    with TileContext(nc) as tc:
        with (
            tc.tile_pool(name="const", bufs=1) as const_pool,
            tc.tile_pool(name="io", bufs=3) as io_pool,
            tc.tile_pool(name="scr", bufs=2) as scr_pool,
            tc.tile_pool(name="psum", bufs=7, space="PSUM") as psum_pool,
            tc.tile_pool(name="psum_warm", bufs=1, space="PSUM") as warm_pool,
        ):
            # const loads go on the ACT ring (empty at start) so the sync
            # ring's first payload load issues immediately.
            coef_sb = const_pool.tile([P, NDIAG * TP], F32)
            nc.scalar.dma_start(coef_sb[:], coef[:])
            cvec_sb = const_pool.tile([P, NDIAG], F32)
            nc.scalar.dma_start(cvec_sb[:], cvec[:])

            def cdiag(k: int, i: int):
                """value_k * I_32 stationary for PE sub-tile row group i."""
                return coef_sb[i * TP : (i + 1) * TP, k * TP : (k + 1) * TP]

            def cval(k: int):
                """value_k as a per-partition scalar operand for the DVE."""
                return cvec_sb[:, k : k + 1]

            # Engine warmups: observe the small constant tiles with a cheap
            # op per engine, so no later instruction needs a multi-sem wait
            # (bacc funnels those through shared event semaphores, which can
            # serialize an engine behind unrelated work).
            warm_ps = warm_pool.tile([P, 2], F32, tag="warm")
            nc.tensor.matmul(warm_ps[:TP], cdiag(0, 0), coef_sb[:TP, 0:2],
                             start=True, stop=True, tile_position=(0, 0))
            warm_v = scr_pool.tile([P, 2], F32, tag="warm_v")
            nc.vector.tensor_copy(warm_v[:], cvec_sb[:, 0:2])

            # ---- bottom pair rows first: these gate PE/DVE compute ----
            # loads split by column half; half 0 (PE's data) lands first so
            # the TensorEngine starts as early as possible.
            b_in = {}
            srcs = (("r1", xr_b1), ("i1", xi_b1), ("r2", xr_b2), ("i2", xi_b2))
            for name, src in srcs:
                b_in[name] = io_pool.tile([P, B], F32, tag=name,
                                          name=f"bin_{name}")
            for c in range(2):
                cs = slice(c * CH, (c + 1) * CH)
                for name, src in srcs:
                    nc.sync.dma_start(b_in[name][:, cs], src[:, cs])

            o_b1 = io_pool.tile([P, 2 * B], F32, tag="o_b1")
            o_b2 = io_pool.tile([P, 2 * B], F32, tag="o_b2")
            o_b = {0: o_b1, 1: o_b2}

            def pe_mix(h, par, cks, cs: slice):
                """one output quantity over column range cs on the PE."""
                n = cs.stop - cs.start
                pt = psum_pool.tile([P, n], F32, tag="ps")
                movs = [b_in[nm][:, cs] for nm in ("r1", "i1", "r2", "i2")]
                for t, (k, mv) in enumerate(zip(cks, movs)):
                    # fp32 matmul costs 4 cyc/moving-column; the four 32x32
                    # diagonal sub-tiles at positions (32i, 32i) execute
                    # concurrently.
                    for i in range(P // TP):
                        nc.tensor.matmul(
                            pt[i * TP : (i + 1) * TP], cdiag(k, i),
                            mv[i * TP : (i + 1) * TP],
                            start=(t == 0), stop=(t == 3),
                            tile_position=(i * TP, i * TP),
                            skip_group_check=True,
                        )
                # ACT sits next to PSUM: interleave-copy PSUM -> SBUF
                nc.scalar.copy(
                    o_b[h][:, 2 * cs.start + par : 2 * cs.stop : 2], pt[:])

            def dve_mix(h, par, cks, cs: slice):
                """one output quantity over column range cs on the DVE."""
                ka, kb, kc, kd = cks
                n = cs.stop - cs.start
                r1 = b_in["r1"][:, cs]
                i1 = b_in["i1"][:, cs]
                r2 = b_in["r2"][:, cs]
                i2 = b_in["i2"][:, cs]
                mul = mybir.AluOpType.mult
                add = mybir.AluOpType.add
                t_a = scr_pool.tile([P, n], F32, tag="ta")
                t_b = scr_pool.tile([P, n], F32, tag="tb")
                # fused (in0 * scalar) + in1 chains: 4 ops per quantity
                nc.vector.tensor_scalar_mul(t_a[:], r1, cval(ka))
                nc.vector.scalar_tensor_tensor(
                    t_b[:], i1, cval(kb), t_a[:], mul, add)
                nc.vector.scalar_tensor_tensor(
                    t_a[:], r2, cval(kc), t_b[:], mul, add)
                nc.vector.scalar_tensor_tensor(
                    o_b[h][:, 2 * cs.start + par : 2 * cs.stop : 2],
                    i2, cval(kd), t_a[:], mul, add)

            c0 = slice(0, CH)
            c1 = slice(CH, B)
            for name, h, par, cks in RECIPES:   # all 4 quantities, half 0
                pe_mix(h, par, cks, c0)
            # o1re half 1 on the PE, psum split in two chunks so its ACT
            # interleave copies (and the dependent store) pipeline.
            pe_mix(*RECIPES[0][1:], slice(CH, CH + CH // 2))
            pe_mix(*RECIPES[0][1:], slice(CH + CH // 2, B))
            for name, h, par, cks in RECIPES[1:]:  # other 3, half 1 -> DVE
                dve_mix(h, par, cks, c1)

            # stores go on the ACT HWDGE ring: HWDGE is FIFO per issuing
            # engine, so a store waiting on compute must not block loads
            # (which are issued on the sync/SP ring and never wait).
            for h, dst in ((0, out_b1), (1, out_b2)):
                for c in range(2):
                    nc.scalar.dma_start(dst[:, c * B : (c + 1) * B],
                                        o_b[h][:, c * B : (c + 1) * B])

            # ---- top rows: identity, just interleave re/im ----
            for b in range(TROWS // P):
                rs = slice(b * P, (b + 1) * P)
                xr_g = io_pool.tile([P, B], F32, tag="xr_top")
                xi_g = io_pool.tile([P, B], F32, tag="xi_top")
                nc.sync.dma_start(xr_g[:], xr_t[rs, :])
                nc.sync.dma_start(xi_g[:], xi_t[rs, :])
                o_g = io_pool.tile([P, 2 * B], F32, tag="out_top")
                nc.vector.tensor_copy(o_g[:, 0 : 2 * B : 2], xr_g[:])
                nc.vector.tensor_copy(o_g[:, 1 : 2 * B : 2], xi_g[:])
                for c in range(2):
                    nc.scalar.dma_start(out_t[rs, c * B : (c + 1) * B],
                                        o_g[:, c * B : (c + 1) * B])

    nc.finalize()
    return nc


_NC_CACHE = None


def _get_nc() -> bacc.Bacc:
    global _NC_CACHE
    if _NC_CACHE is None:
        _NC_CACHE = _build_nc()
    return _NC_CACHE


def _coef_values(M_re: np.ndarray, M_im: np.ndarray):
    """Host-side 2x2 expm of the anti-Hermitian generator -> coef arrays."""
    M = M_re.astype(np.float64) + 1j * M_im.astype(np.float64)
    A = M - M.conj().T          # anti-Hermitian
    H = -1j * A                 # Hermitian
    w, V = np.linalg.eigh(H)
    Mexp = V @ np.diag(np.exp(1j * w)) @ V.conj().T   # expm(A), exact
    c00, c01 = Mexp[0, 0], Mexp[0, 1]
    c10, c11 = Mexp[1, 0], Mexp[1, 1]
    vals = [
        c00.real, -c00.imag, c00.imag,
        c01.real, -c01.imag, c01.imag,
        c10.real, -c10.imag, c10.imag,
        c11.real, -c11.imag, c11.imag,
    ]
    coef = np.zeros((P, NDIAG * TP), dtype=np.float32)
    idx = np.arange(P)
    for k, v in enumerate(vals):
        coef[idx, k * TP + (idx % TP)] = np.float32(v)
    cvec = np.tile(np.array(vals, dtype=np.float32), (P, 1))
    return coef, cvec


def _in_map(x_re, x_im, coef, cvec, d: int) -> dict:
    t0 = d * TROWS
    b1 = D // 2 + d * PROWS
    b2 = 3 * D // 4 + d * PROWS
    return {
        "xr_t": x_re[t0 : t0 + TROWS],
        "xi_t": x_im[t0 : t0 + TROWS],
        "xr_b1": x_re[b1 : b1 + PROWS],
        "xi_b1": x_im[b1 : b1 + PROWS],
        "xr_b2": x_re[b2 : b2 + PROWS],
        "xi_b2": x_im[b2 : b2 + PROWS],
        "coef": coef,
        "cvec": cvec,
    }


def kernel(M_re, M_im, x_re, x_im) -> np.ndarray:
    M_re = np.asarray(M_re, dtype=np.float32)
    M_im = np.asarray(M_im, dtype=np.float32)
    x_re = np.ascontiguousarray(x_re, dtype=np.float32)
    x_im = np.ascontiguousarray(x_im, dtype=np.float32)

    coef, cvec = _coef_values(M_re, M_im)
    in_maps = [_in_map(x_re, x_im, coef, cvec, d) for d in range(NCORES)]

    nc = _get_nc()
    res = run_bass_kernel_spmd(nc, in_maps, core_ids=list(range(NCORES)))

    full = np.empty((D, 2 * B), dtype=np.float32)
    for d, r in enumerate(res.results):
        t0 = d * TROWS
        b1 = D // 2 + d * PROWS
        b2 = 3 * D // 4 + d * PROWS
        full[t0 : t0 + TROWS] = r["out_t"]
        full[b1 : b1 + PROWS] = r["out_b1"]
        full[b2 : b2 + PROWS] = r["out_b2"]
    return full.view(np.complex64)  # (4096, 1024)



# revision 10
# speedup vs baseline: 1.1053x; 1.1053x over previous
"""Trainium2 Bass kernel for the controlled-U (CU) gate application.

Math: the reference builds U = P0 (x) I (x) ... + P1 (x) Mexp (x) I ...
with dim=2, wires=12, index=(0,1), control_state=(1,). This factors as

    U = diag(I_2048, Mexp (x) I_1024)        (4096 x 4096)

so U @ x is:
    out[0:2048]     = x[0:2048]                        (identity)
    out[2048:3072]  = c00 * x[2048:3072] + c01 * x[3072:4096]
    out[3072:4096]  = c10 * x[2048:3072] + c11 * x[3072:4096]

with [[c00, c01], [c10, c11]] = Mexp = expm(M - M^H), a 2x2 unitary
computed exactly on host (eigendecomposition of the 2x2 Hermitian
generator).

Device strategy (8 NeuronCores, SPMD row sharding; the kernel is
HBM-bandwidth bound at ~8 MiB DMA per core):
  - core d gets top rows [256d, 256d+256) (identity) plus the bottom
    pair rows [2048+128d, +128) and [3072+128d, +128) (the 2x2 mix).
  - loads ride the sync (SP) HWDGE ring back-to-back: bottom rows
    first (they gate compute), split by column halves so compute and
    the first stores start early; top rows after.
  - bottom mix = 4 output quantities, each sum_k coef_k * in_k:
      o1 (rows 2048..3071) -> DVE: 1 tensor_scalar + 3 fused
        scalar_tensor_tensor MACs per column half, the last MAC
        writing the interleaved (re,im) layout directly.
      o2 (rows 3072..4095) -> TensorE as SINGLE-PASS fp32r matmuls
        (f32 tiles bitcast to float32r; 32x32 diagonal stationary
        tiles at concurrent tile positions, PSUM accumulation over
        the 4 inputs) + ACT interleave-copies PSUM -> SBUF.
    fp32r drops mantissa bits in the multiply (~1e-3 rel) - far
    inside the 2e-2 gate - and avoids fp32's LOW_HIGH double-pass
    emulation that made the PE the critical path.
  - top rows: ACT interleave-copies re/im and issues the top stores
    on its own (ACT) HWDGE ring; bottom stores ride the SP ring
    after all loads, issued per column half as soon as ready.
  - outputs per core: f32 rows of interleaved (re, im) pairs; the
    host reassembles (4096, 2048) f32 and reinterprets as complex64.
"""

import numpy as np

import concourse.bacc as bacc
import concourse.mybir as mybir
from concourse.tile import TileContext
from concourse.bass_utils import run_bass_kernel_spmd

# Problem geometry (hardcoded per the task contract).
D = 4096           # state dimension 2**12
B = 1024           # batch
NCORES = 8
P = 128            # SBUF partitions
TROWS = D // 2 // NCORES   # 256 top (identity) rows per core
PROWS = D // 4 // NCORES   # 128 bottom pair rows per core
F32 = mybir.dt.float32
F32R = mybir.dt.float32r

NDIAG = 12         # 12 coefficient scalars (signs baked in)
NPED = 6           # distinct coefficients used by the PE (o2) quantities
CH = B // 2        # column half

# quantity -> (out half, interleave parity, coefficient idx per input).
# inputs are (xr1, xi1, xr2, xi2); coefficients include baked-in signs.
RECIPES = [
    ("o1re", 0, 0, (0, 1, 3, 4)),
    ("o1im", 0, 1, (2, 0, 5, 3)),
    ("o2re", 1, 0, (6, 7, 9, 10)),
    ("o2im", 1, 1, (8, 6, 11, 9)),
]


def _build_nc() -> bacc.Bacc:
    """Build the per-core Bass/Tile program (identical on all 8 cores)."""
    nc = bacc.Bacc("TRN2", enable_partition_id=False)

    xr_t = nc.dram_tensor("xr_t", [TROWS, B], F32, kind="ExternalInput")
    xi_t = nc.dram_tensor("xi_t", [TROWS, B], F32, kind="ExternalInput")
    # bottom inputs + coef are declared float32r end-to-end (same bits as
    # f32) so the BIR verifier accepts them as fp32r matmul operands; the
    # DVE chains bitcast them back to f32.
    xr_b1 = nc.dram_tensor("xr_b1", [PROWS, B], F32R, kind="ExternalInput")
    xi_b1 = nc.dram_tensor("xi_b1", [PROWS, B], F32R, kind="ExternalInput")
    xr_b2 = nc.dram_tensor("xr_b2", [PROWS, B], F32R, kind="ExternalInput")
    xi_b2 = nc.dram_tensor("xi_b2", [PROWS, B], F32R, kind="ExternalInput")
    # coef[p, j*P + p] = value_{6+j}  ->  128x128 diagonal blocks (the
    # fp32r matmul path requires full-array matmuls, no tile_position).
    coef = nc.dram_tensor("coef", [P, NPED * P], F32R, kind="ExternalInput")
    cvec = nc.dram_tensor("cvec", [P, NDIAG], F32, kind="ExternalInput")

    out_t = nc.dram_tensor("out_t", [TROWS, 2 * B], F32, kind="ExternalOutput")
    out_b1 = nc.dram_tensor("out_b1", [PROWS, 2 * B], F32, kind="ExternalOutput")
    out_b2 = nc.dram_tensor("out_b2", [PROWS, 2 * B], F32, kind="ExternalOutput")

    mul = mybir.AluOpType.mult
    add = mybir.AluOpType.add

    with TileContext(nc) as tc:
        with (
            tc.tile_pool(name="const", bufs=1) as const_pool,
            tc.tile_pool(name="io", bufs=1) as io_pool,
            tc.tile_pool(name="scr", bufs=1) as scr_pool,
            tc.tile_pool(name="psum", bufs=1, space="PSUM") as psum_pool,
        ):
            # tiny cvec first on the SP ring (gates DVE/ACT warmups).
            cvec_sb = const_pool.tile([P, NDIAG], F32)
            nc.sync.dma_start(cvec_sb[:], cvec[:])
            coef_sb = const_pool.tile([P, NPED * P], F32R)

            def cval(k: int):
                """value_k as a per-partition scalar operand."""
                return cvec_sb[:, k : k + 1]

            def cdiag(k: int):
                """value_k * I_128 stationary (k is a RECIPES coef index)."""
                j = k - 6
                return coef_sb[:, j * P : (j + 1) * P]

            # Engine warmups: observe the constant tiles with a cheap op
            # per engine so no payload instruction needs a multi-sem wait,
            # and so ACT's table load happens up front.
            warm_ps = psum_pool.tile([P, 2], F32, tag="warm")
            nc.tensor.matmul(warm_ps[:], cdiag(6), coef_sb[:, 0:2],
                             start=True, stop=True)
            warm_v = scr_pool.tile([P, 2], F32, tag="warm_v")
            nc.vector.tensor_copy(warm_v[:], cvec_sb[:, 0:2])
            warm_a = scr_pool.tile([P, 2], F32, tag="warm_a")
            nc.scalar.copy(warm_a[:], cvec_sb[:, 0:2])

            # ---- loads: bottom pair rows first, full 4 KiB rows ----
            # (few big DMAs: the SP sequencer spends ~0.7 us issuing each
            # DMA instruction, so many small loads starve the DMA engines)
            b_in = {}
            srcs = (("r1", xr_b1), ("i1", xi_b1), ("r2", xr_b2), ("i2", xi_b2))
            for name, src in srcs:
                b_in[name] = io_pool.tile([P, B], F32R, tag=f"b_{name}",
                                          name=f"bin_{name}")
            for name, src in srcs:
                nc.sync.dma_start(b_in[name][:], src[:])
                if name == "i1":
                    # coef (384 KiB) lands mid-way: before the PE needs it,
                    # without delaying the first bottom tiles
                    nc.sync.dma_start(coef_sb[:], coef[:])
            t_in = []
            for b in range(TROWS // P):
                rs = slice(b * P, (b + 1) * P)
                tr = io_pool.tile([P, B], F32, tag=f"t_r{b}")
                ti = io_pool.tile([P, B], F32, tag=f"t_i{b}")
                nc.sync.dma_start(tr[:], xr_t[rs, :])
                nc.sync.dma_start(ti[:], xi_t[rs, :])
                t_in.append((tr, ti))

            o_b1 = io_pool.tile([P, 2 * B], F32, tag="o_b1")
            o_b2 = io_pool.tile([P, 2 * B], F32, tag="o_b2")
            o_b = {0: o_b1, 1: o_b2}

            def pe_mm(h, par, cks, cs):
                """matmuls for one o2 quantity over column range cs (PE)."""
                n = cs.stop - cs.start
                pt = psum_pool.tile([P, CH], F32, tag=f"ps{par}_{cs.start}",
                                    name=f"ps{par}_{cs.start}")
                movs = [b_in[nm][:, cs] for nm in ("r1", "i1", "r2", "i2")]
                for t, (k, mv) in enumerate(zip(cks, movs)):
                    nc.tensor.matmul(pt[:, :n], cdiag(k), mv,
                                     start=(t == 0), stop=(t == 3))
                return pt

            def act_evac(h, par, cs, pt):
                """interleave-copy PSUM -> SBUF on ACT."""
                nc.scalar.copy(
                    o_b[h][:, 2 * cs.start + par : 2 * cs.stop : 2],
                    pt[:, : cs.stop - cs.start])

            def dve_mix(h, par, cks, cs):
                """one quantity over column range cs on the DVE."""
                ka, kb, kc, kd = cks
                n = cs.stop - cs.start
                ta = scr_pool.tile([P, CH], F32, tag=f"ta{par}_{cs.start}",
                                   name=f"ta{par}_{cs.start}")
                tb = scr_pool.tile([P, CH], F32, tag=f"tb{par}_{cs.start}",
                                   name=f"tb{par}_{cs.start}")
                nc.vector.tensor_scalar_mul(
                    ta[:, :n], b_in["r1"][:, cs].bitcast(F32), cval(ka))
                nc.vector.scalar_tensor_tensor(
                    tb[:, :n], b_in["i1"][:, cs].bitcast(F32), cval(kb),
                    ta[:, :n], mul, add)
                nc.vector.scalar_tensor_tensor(
                    ta[:, :n], b_in["r2"][:, cs].bitcast(F32), cval(kc),
                    tb[:, :n], mul, add)
                nc.vector.scalar_tensor_tensor(
                    o_b[h][:, 2 * cs.start + par : 2 * cs.stop : 2],
                    b_in["i2"][:, cs].bitcast(F32), cval(kd), ta[:, :n],
                    mul, add)

            h0 = slice(0, CH)
            h1 = slice(CH, B)
            # PE streams all 16 matmuls back-to-back (no idle gaps -> HAM
            # stays warm); DVE runs its 4 half-chains independently.
            pts = {}
            for cs in (h0, h1):
                pts[("re", cs.start)] = pe_mm(*RECIPES[2][1:], cs)
                pts[("im", cs.start)] = pe_mm(*RECIPES[3][1:], cs)
            for cs in (h0, h1):
                dve_mix(*RECIPES[0][1:], cs)
                dve_mix(*RECIPES[1][1:], cs)

            # ACT program order = issue order: evacs+store for half 0,
            # top block 0, evacs+store for half 1, top block 1.  Each
            # out_b2/out_t store follows its producer on the same engine
            # (FIFO, no cross-engine sem hop).
            for i, cs in enumerate((h0, h1)):
                act_evac(1, 0, cs, pts[("re", cs.start)])
                act_evac(1, 1, cs, pts[("im", cs.start)])
                ds = slice(2 * cs.start, 2 * cs.stop)
                nc.scalar.dma_start(out_b2[:, ds], o_b2[:, ds])
                tr, ti = t_in[i]
                rs = slice(i * P, (i + 1) * P)
                o_g = io_pool.tile([P, 2 * B], F32, tag=f"out_top{i}",
                                   name=f"out_top{i}")
                nc.scalar.copy(o_g[:, 0 : 2 * B : 2], tr[:])
                nc.scalar.copy(o_g[:, 1 : 2 * B : 2], ti[:])
                nc.scalar.dma_start(out_t[rs, :], o_g[:])

            # ---- o1 bottom stores: SP ring, per column half ----
            for c in range(2):
                ds = slice(c * B, (c + 1) * B)
                nc.sync.dma_start(out_b1[:, ds], o_b1[:, ds])

    nc.finalize()
    return nc


_NC_CACHE = None


def _get_nc() -> bacc.Bacc:
    global _NC_CACHE
    if _NC_CACHE is None:
        _NC_CACHE = _build_nc()
    return _NC_CACHE


def _coef_values(M_re: np.ndarray, M_im: np.ndarray):
    """Host-side 2x2 expm of the anti-Hermitian generator -> coef arrays."""
    M = M_re.astype(np.float64) + 1j * M_im.astype(np.float64)
    A = M - M.conj().T          # anti-Hermitian
    H = -1j * A                 # Hermitian
    w, V = np.linalg.eigh(H)
    Mexp = V @ np.diag(np.exp(1j * w)) @ V.conj().T   # expm(A), exact
    c00, c01 = Mexp[0, 0], Mexp[0, 1]
    c10, c11 = Mexp[1, 0], Mexp[1, 1]
    vals = [
        c00.real, -c00.imag, c00.imag,
        c01.real, -c01.imag, c01.imag,
        c10.real, -c10.imag, c10.imag,
        c11.real, -c11.imag, c11.imag,
    ]
    coef = np.zeros((P, NPED * P), dtype=np.float32)
    idx = np.arange(P)
    for j in range(NPED):
        coef[idx, j * P + idx] = np.float32(vals[6 + j])
    cvec = np.tile(np.array(vals, dtype=np.float32), (P, 1))
    return coef, cvec


def _in_map(x_re, x_im, coef, cvec, d: int) -> dict:
    t0 = d * TROWS
    b1 = D // 2 + d * PROWS
    b2 = 3 * D // 4 + d * PROWS
    return {
        "xr_t": x_re[t0 : t0 + TROWS],
        "xi_t": x_im[t0 : t0 + TROWS],
        "xr_b1": x_re[b1 : b1 + PROWS],
        "xi_b1": x_im[b1 : b1 + PROWS],
        "xr_b2": x_re[b2 : b2 + PROWS],
        "xi_b2": x_im[b2 : b2 + PROWS],
        "coef": coef,
        "cvec": cvec,
    }


def kernel(M_re, M_im, x_re, x_im) -> np.ndarray:
    M_re = np.asarray(M_re, dtype=np.float32)
    M_im = np.asarray(M_im, dtype=np.float32)
    x_re = np.ascontiguousarray(x_re, dtype=np.float32)
    x_im = np.ascontiguousarray(x_im, dtype=np.float32)

    coef, cvec = _coef_values(M_re, M_im)
    in_maps = [_in_map(x_re, x_im, coef, cvec, d) for d in range(NCORES)]

    nc = _get_nc()
    res = run_bass_kernel_spmd(nc, in_maps, core_ids=list(range(NCORES)))

    full = np.empty((D, 2 * B), dtype=np.float32)
    for d, r in enumerate(res.results):
        t0 = d * TROWS
        b1 = D // 2 + d * PROWS
        b2 = 3 * D // 4 + d * PROWS
        full[t0 : t0 + TROWS] = r["out_t"]
        full[b1 : b1 + PROWS] = r["out_b1"]
        full[b2 : b2 + PROWS] = r["out_b2"]
    return full.view(np.complex64)  # (4096, 1024)


# revision 14
# speedup vs baseline: 1.3189x; 1.1932x over previous
"""Trainium2 Bass kernel for the controlled-U (CU) gate application.

Math: the reference builds U = P0 (x) I (x) ... + P1 (x) Mexp (x) I ...
with dim=2, wires=12, index=(0,1), control_state=(1,). This factors as

    U = diag(I_2048, Mexp (x) I_1024)        (4096 x 4096)

so U @ x is:
    out[0:2048]     = x[0:2048]                        (identity)
    out[2048:3072]  = c00 * x[2048:3072] + c01 * x[3072:4096]
    out[3072:4096]  = c10 * x[2048:3072] + c11 * x[3072:4096]

with [[c00, c01], [c10, c11]] = Mexp = expm(M - M^H), a 2x2 unitary
computed exactly on host (eigendecomposition of the 2x2 Hermitian
generator).

The kernel is pure data movement + a broadcast 2x2 mix, so it is HBM
bandwidth bound.  The device stages everything in bf16 (the harness
gate is rel_err < 2e-2; the bf16 pipeline measures ~2.5e-3), which
halves DMA traffic to ~4.3 MiB per core vs f32 staging:

  - 8 cores, SPMD row sharding: core d gets top rows [256d, 256d+256)
    (identity) plus the bottom pair rows [2048+128d, +128) and
    [3072+128d, +128) (the 2x2 mix).
  - host pre-packs bf16 inputs as column-concatenated pairs
    ([r1|i1], [r2|i2], [tr|ti] per top block) so the load stream is
    few large DMAs (the SP sequencer spends ~0.7 us ISSUING each DMA
    instruction; many small loads would starve the DMA engines).
  - bottom mix = 4 output quantities, each sum_k coef_k * in_k:
      o1 -> DVE: 1 tensor_scalar + 3 fused scalar_tensor_tensor MACs
        in bf16 (2x/4x DVE perf modes), the last MAC writing the
        interleaved (re,im) layout directly.
      o2 -> TensorE: bf16 matmuls with c_k * I_128 diagonal
        stationaries, 1024-column moving operands, f32 PSUM
        accumulation over the 4 inputs; ACT interleave-copies
        PSUM -> SBUF (bf16 cast).
  - top rows: ACT interleave-copies re/im; out_t stores issue on the
    ACT HWDGE ring right after (same-engine FIFO, no sem hop);
    out_b1 stores ride the SP ring after all loads.
  - outputs per core: bf16 rows of interleaved (re, im) pairs; the
    host upcasts to f32 and reinterprets as complex64.
"""

import ml_dtypes
import numpy as np

import concourse.bacc as bacc
import concourse.mybir as mybir
from concourse.tile import TileContext
from concourse.bass_utils import run_bass_kernel_spmd

# Problem geometry (hardcoded per the task contract).
D = 4096           # state dimension 2**12
B = 1024           # batch
NCORES = 8
P = 128            # SBUF partitions
TROWS = D // 2 // NCORES   # 256 top (identity) rows per core
PROWS = D // 4 // NCORES   # 128 bottom pair rows per core
F32 = mybir.dt.float32
BF16 = mybir.dt.bfloat16
NPBF = ml_dtypes.bfloat16

NDIAG = 12         # 12 coefficient scalars (signs baked in)
NPED = 6           # distinct coefficients used by the PE (o2) quantities

# quantity -> (out half, interleave parity, coefficient idx per input).
# inputs are (xr1, xi1, xr2, xi2); coefficients include baked-in signs.
RECIPES = [
    ("o1re", 0, 0, (0, 1, 3, 4)),
    ("o1im", 0, 1, (2, 0, 5, 3)),
    ("o2re", 1, 0, (6, 7, 9, 10)),
    ("o2im", 1, 1, (8, 6, 11, 9)),
]


def _build_nc() -> bacc.Bacc:
    """Build the per-core Bass/Tile program (identical on all 8 cores)."""
    nc = bacc.Bacc("TRN2", enable_partition_id=False)

    # column-concatenated bf16 input pairs (packed on host)
    xb_a = nc.dram_tensor("xb_a", [PROWS, 2 * B], BF16, kind="ExternalInput")
    xb_b = nc.dram_tensor("xb_b", [PROWS, 2 * B], BF16, kind="ExternalInput")
    xt0 = nc.dram_tensor("xt0", [P, 2 * B], BF16, kind="ExternalInput")
    xt1 = nc.dram_tensor("xt1", [P, 2 * B], BF16, kind="ExternalInput")
    # coef[p, j*P + p] = value_{6+j}  ->  128x128 diagonal blocks.
    coef = nc.dram_tensor("coef", [P, NPED * P], BF16, kind="ExternalInput")
    cvec = nc.dram_tensor("cvec", [P, NDIAG], F32, kind="ExternalInput")

    out_t = nc.dram_tensor("out_t", [TROWS, 2 * B], BF16,
                           kind="ExternalOutput")
    out_b1 = nc.dram_tensor("out_b1", [PROWS, 2 * B], BF16,
                            kind="ExternalOutput")
    out_b2 = nc.dram_tensor("out_b2", [PROWS, 2 * B], BF16,
                            kind="ExternalOutput")

    mul = mybir.AluOpType.mult
    add = mybir.AluOpType.add

    with TileContext(nc) as tc:
        with (
            tc.tile_pool(name="const", bufs=1) as const_pool,
            tc.tile_pool(name="io", bufs=1) as io_pool,
            tc.tile_pool(name="scr", bufs=1) as scr_pool,
            tc.tile_pool(name="psum", bufs=1, space="PSUM") as psum_pool,
        ):
            # tiny cvec first on the SP ring (gates DVE/ACT warmups).
            cvec_sb = const_pool.tile([P, NDIAG], F32)
            nc.sync.dma_start(cvec_sb[:], cvec[:])
            coef_sb = const_pool.tile([P, NPED * P], BF16)

            def cval(k: int):
                """value_k as a per-partition scalar operand."""
                return cvec_sb[:, k : k + 1]

            def cdiag(k: int):
                """value_k * I_128 stationary (k is a RECIPES coef index)."""
                j = k - 6
                return coef_sb[:, j * P : (j + 1) * P]

            # ---- loads: bottom pair rows first, then coef, then top ----
            ba = io_pool.tile([P, 2 * B], BF16, tag="ba")
            bb = io_pool.tile([P, 2 * B], BF16, tag="bb")
            nc.sync.dma_start(ba[:], xb_a[:])
            nc.sync.dma_start(coef_sb[:], coef[:])
            nc.sync.dma_start(bb[:], xb_b[:])
            t_in = []
            for b, src in enumerate((xt0, xt1)):
                tt = io_pool.tile([P, 2 * B], BF16, tag=f"t_{b}",
                                  name=f"tin_{b}")
                nc.sync.dma_start(tt[:], src[:])
                t_in.append(tt)

            b_in = {
                "r1": ba[:, 0:B], "i1": ba[:, B : 2 * B],
                "r2": bb[:, 0:B], "i2": bb[:, B : 2 * B],
            }

            # Engine warmups: observe the constant tiles with a cheap op
            # per engine so no payload instruction needs a multi-sem wait,
            # and so ACT's table load happens up front.
            warm_ps = psum_pool.tile([P, 2], F32, tag="warm")
            nc.tensor.matmul(warm_ps[:], cdiag(6), coef_sb[:, 0:2],
                             start=True, stop=True)
            warm_v = scr_pool.tile([P, 2], F32, tag="warm_v")
            nc.vector.tensor_copy(warm_v[:], cvec_sb[:, 0:2])
            warm_a = scr_pool.tile([P, 2], F32, tag="warm_a")
            nc.scalar.copy(warm_a[:], cvec_sb[:, 0:2])

            o_b1 = io_pool.tile([P, 2 * B], BF16, tag="o_b1")
            o_b2 = io_pool.tile([P, 2 * B], BF16, tag="o_b2")
            o_b = {0: o_b1, 1: o_b2}

            def dve_mix(h, par, cks):
                """one full-width quantity on the DVE (bf16 chain)."""
                ka, kb, kc, kd = cks
                ta = scr_pool.tile([P, B], BF16, tag=f"ta{par}",
                                   name=f"ta{par}")
                tb = scr_pool.tile([P, B], BF16, tag=f"tb{par}",
                                   name=f"tb{par}")
                nc.vector.tensor_scalar_mul(ta[:], b_in["r1"], cval(ka))
                nc.vector.scalar_tensor_tensor(
                    tb[:], b_in["i1"], cval(kb), ta[:], mul, add)
                nc.vector.scalar_tensor_tensor(
                    ta[:], b_in["r2"], cval(kc), tb[:], mul, add)
                nc.vector.scalar_tensor_tensor(
                    o_b[h][:, par : 2 * B : 2],
                    b_in["i2"], cval(kd), ta[:], mul, add)

            # PE: o2re then o2im, per 512-column half (one PSUM bank per
            # matmul output), 16 bf16 matmuls total
            CH = B // 2
            pts = {}
            for c in range(2):
                cs = slice(c * CH, (c + 1) * CH)
                for name, h, par, cks in RECIPES[2:]:
                    pt = psum_pool.tile([P, CH], F32, tag=f"ps{par}_{c}",
                                        name=f"ps{par}_{c}")
                    for t, (k, nm) in enumerate(
                            zip(cks, ("r1", "i1", "r2", "i2"))):
                        nc.tensor.matmul(pt[:], cdiag(k), b_in[nm][:, cs],
                                         start=(t == 0), stop=(t == 3))
                    pts[(par, c)] = pt

            # DVE: o1re, o1im chains
            dve_mix(*RECIPES[0][1:])
            dve_mix(*RECIPES[1][1:])

            # ACT program order = issue order: top block 0 (+store),
            # o2 PSUM evacs per half (+store), top block 1 (+store).
            def act_evac_half(c):
                cs = slice(c * CH, (c + 1) * CH)
                nc.scalar.copy(
                    o_b2[:, 2 * cs.start + 0 : 2 * cs.stop : 2],
                    pts[(0, c)][:])
                nc.scalar.copy(
                    o_b2[:, 2 * cs.start + 1 : 2 * cs.stop : 2],
                    pts[(1, c)][:])
                ds = slice(2 * cs.start, 2 * cs.stop)
                nc.scalar.dma_start(out_b2[:, ds], o_b2[:, ds])

            for b in (0, 1):
                tt = t_in[b]
                rs = slice(b * P, (b + 1) * P)
                o_g = io_pool.tile([P, 2 * B], BF16, tag=f"out_top{b}",
                                   name=f"out_top{b}")
                nc.scalar.copy(o_g[:, 0 : 2 * B : 2], tt[:, 0:B])
                nc.scalar.copy(o_g[:, 1 : 2 * B : 2], tt[:, B : 2 * B])
                nc.scalar.dma_start(out_t[rs, :], o_g[:])
                act_evac_half(b)

            # ---- o1 bottom store: SP ring, after all loads ----
            nc.sync.dma_start(out_b1[:], o_b1[:])

    nc.finalize()
    return nc


_NC_CACHE = None


def _get_nc() -> bacc.Bacc:
    global _NC_CACHE
    if _NC_CACHE is None:
        _NC_CACHE = _build_nc()
    return _NC_CACHE


def _coef_values(M_re: np.ndarray, M_im: np.ndarray):
    """Host-side 2x2 expm of the anti-Hermitian generator -> coef arrays."""
    M = M_re.astype(np.float64) + 1j * M_im.astype(np.float64)
    A = M - M.conj().T          # anti-Hermitian
    H = -1j * A                 # Hermitian
    w, V = np.linalg.eigh(H)
    Mexp = V @ np.diag(np.exp(1j * w)) @ V.conj().T   # expm(A), exact
    c00, c01 = Mexp[0, 0], Mexp[0, 1]
    c10, c11 = Mexp[1, 0], Mexp[1, 1]
    vals = [
        c00.real, -c00.imag, c00.imag,
        c01.real, -c01.imag, c01.imag,
        c10.real, -c10.imag, c10.imag,
        c11.real, -c11.imag, c11.imag,
    ]
    coef = np.zeros((P, NPED * P), dtype=NPBF)
    idx = np.arange(P)
    for j in range(NPED):
        coef[idx, j * P + idx] = NPBF(vals[6 + j])
    cvec = np.tile(np.array(vals, dtype=np.float32), (P, 1))
    return coef, cvec


def _in_map(x_re, x_im, coef, cvec, d: int) -> dict:
    """Per-core input dict; casts the core's slices to bf16 and packs
    column-concatenated pairs."""
    t0 = d * TROWS
    b1 = D // 2 + d * PROWS
    b2 = 3 * D // 4 + d * PROWS

    def bf(a):
        return np.ascontiguousarray(a).astype(NPBF)

    return {
        "xb_a": np.concatenate(
            [bf(x_re[b1 : b1 + PROWS]), bf(x_im[b1 : b1 + PROWS])], axis=1),
        "xb_b": np.concatenate(
            [bf(x_re[b2 : b2 + PROWS]), bf(x_im[b2 : b2 + PROWS])], axis=1),
        "xt0": np.concatenate(
            [bf(x_re[t0 : t0 + P]), bf(x_im[t0 : t0 + P])], axis=1),
        "xt1": np.concatenate(
            [bf(x_re[t0 + P : t0 + TROWS]), bf(x_im[t0 + P : t0 + TROWS])],
            axis=1),
        "coef": coef,
        "cvec": cvec,
    }


def kernel(M_re, M_im, x_re, x_im) -> np.ndarray:
    M_re = np.asarray(M_re, dtype=np.float32)
    M_im = np.asarray(M_im, dtype=np.float32)
    x_re = np.ascontiguousarray(x_re, dtype=np.float32)
    x_im = np.ascontiguousarray(x_im, dtype=np.float32)

    coef, cvec = _coef_values(M_re, M_im)
    in_maps = [_in_map(x_re, x_im, coef, cvec, d) for d in range(NCORES)]

    nc = _get_nc()
    res = run_bass_kernel_spmd(nc, in_maps, core_ids=list(range(NCORES)))

    full = np.empty((D, 2 * B), dtype=np.float32)
    for d, r in enumerate(res.results):
        t0 = d * TROWS
        b1 = D // 2 + d * PROWS
        b2 = 3 * D // 4 + d * PROWS
        full[t0 : t0 + TROWS] = np.asarray(r["out_t"]).astype(np.float32)
        full[b1 : b1 + PROWS] = np.asarray(r["out_b1"]).astype(np.float32)
        full[b2 : b2 + PROWS] = np.asarray(r["out_b2"]).astype(np.float32)
    return full.view(np.complex64)  # (4096, 1024)
